# revision 58
# baseline (speedup 1.0000x reference)
"""MoE kernel for Trainium2 (8 NeuronCores, expert-parallel, load-balanced).

Strategy
--------
N=8192 tokens, D=1024, E=8 experts, DFF=4096, top_k=2. The reference
computes every expert densely and masks; only each token's top-2 experts
contribute, so we dispatch each token to its 2 experts and run the
expert MLPs on just the routed tokens: 4x fewer FLOPs than dense.

Load balance: expert loads are uneven (1932..2182 here), so instead of
one expert per core (which pads every core to the straggler's 2304
tokens), each core runs K weight slots with compile-time sizes
sum(sizes)=S. The host solves a small covering problem (DP) for the
minimal S such that all experts' token lists pack into 8 bins per slot
class (each bin single-expert); K=3 lands at S=2064 vs the perfect
2048 vs the naive 2304 (-10% PE time).

Both matmuls keep tokens on the PE free dim (phase A: h^T[f,t], phase
B: y^T[d,t]), so block sizes are exact token counts - no 128-row
padding anywhere. The gate weight is applied with a DVE elementwise
multiply against a partition-broadcast copy of the combine weights
(free-dim scaling can't use the activation-unit per-partition scale).

bf16 storage end-to-end; weights are loaded into SBUF once per slot
(graded chunk sizes in consumption order: small first so the first
matmuls start ~5us in, large after for DMA efficiency; each next
slot's load overlaps the previous slot's trailing phase B). Phase A of
the first two blocks is fused per-mf so the w1 stream keeps up.

Host (unshard): y[token] = yT[core1][:, col1] + yT[core2][:, col2].
"""

import numpy as np

import concourse.bass as bass
import concourse.bacc as bacc
import concourse.tile as tile
from concourse import mybir
from concourse.bass_utils import run_bass_kernel_spmd

N, D, E, DFF = 8192, 1024, 8, 4096
P = 128
KD = D // P  # 8 k-chunks, first matmul
KF = DFF // P  # 32 k-chunks, second matmul
MD = D // P  # 8 output-row tiles, second matmul

TRACE = False
LAST_RESULT = None
LAST_NC = None
REPS = 1  # >1: repeat whole computation in-program (for slope timing)


def _chunks_even(total, maxb=512):
    """Split into near-equal blocks <= maxb, multiples of 16 (except possibly
    the last), avoiding tiny tail blocks that expose handoff latency."""
    nb = -(-total // maxb)
    out, rem = [], total
    for i in range(nb):
        b = min(rem, int(np.ceil(rem / (nb - i) / 16) * 16), maxb)
        out.append(b)
        rem -= b
    assert rem == 0 and sum(out) == total
    return out


def _feasible(counts, sizes, n_bins=E, parents=None):
    """DP: can counts be covered with n_bins bins of each size class?
    State: per-class bins used. If parents given, fill for backtracking."""
    K = len(sizes)
    reach = {tuple([0] * K)}
    for e, c in enumerate(counts):
        nxt = set()
        pe = {} if parents is not None else None
        for st in reach:

            def rec(k, st_k, rem):
                if rem <= 0:
                    key = tuple(st_k)
                    if key not in nxt:
                        nxt.add(key)
                        if pe is not None:
                            pe[key] = (st, tuple(np.subtract(st_k, st)))
                    return
                if k == K:
                    return
                for nk in range(n_bins - st_k[k] + 1):
                    st2 = list(st_k)
                    st2[k] += nk
                    rec(k + 1, st2, rem - nk * sizes[k])
                    if nk * sizes[k] >= rem:
                        break

            rec(0, list(st), c)
        if parents is not None:
            parents.append(pe)
        reach = nxt
        if not reach:
            return None
    return next(iter(reach))


def _optimize_slots(counts, n_bins=E):
    """Find slot sizes (K=2, or 3 if strictly better) minimizing
    S = sum(sizes). Returns (sizes, assign) with assign[e][k] = #bins of
    class k used by expert e."""
    counts = np.asarray(counts, dtype=int)
    lo = int(np.ceil(counts.sum() / n_bins / 16) * 16)
    hi = int(np.ceil(counts.max() / 16) * 16) + 16

    def slack_ok(S):
        # zero-slack S needs an exact cover by multiples of 16 => every
        # count must be divisible by 16 (cheap prune of the full scan)
        slack = n_bins * S - int(counts.sum())
        return slack > 0 or all(c % 16 == 0 for c in counts)

    best = None
    S2 = None
    for S in range(lo, 2 * hi, 16):
        if not slack_ok(S):
            continue
        for S_A in range(256, S // 2 + 1, 16):
            if _feasible(counts, (S_A, S - S_A)) is not None:
                best = (S_A, S - S_A)
                break
        if best:
            S2 = S
            break
    assert best is not None, "no 2-slot split found"

    found3 = None
    for S in range(lo, S2, 16):
        if not slack_ok(S):
            continue
        for S_A in range(256, S // 3 + 1, 16):
            for S_B in range(S_A, (S - S_A) // 2 + 1, 16):
                S_C = S - S_A - S_B
                if _feasible(counts, (S_A, S_B, S_C)) is not None:
                    found3 = (S_A, S_B, S_C)
                    break
            if found3:
                break
        if found3:
            break
    sizes = found3 if found3 is not None else best

    # Order slots to maximize the weight-reload windows: the reload of slot
    # k+1 overlaps slot k's LAST block's phase B, so prefer large last
    # blocks on the slots that precede a reload (and a large first slot for
    # the fused start).
    import itertools

    def min_window(order):
        wins = [_chunks_even(order[k])[-1] for k in range(len(order) - 1)]
        return min(wins) if wins else 1 << 30

    sizes = max(
        itertools.permutations(sizes), key=lambda o: (min_window(o), o[0])
    )

    parents = []
    assert _feasible(counts, sizes, n_bins, parents) is not None
    assign = [None] * len(counts)
    cur = next(iter(parents[-1]))
    for e in range(len(counts) - 1, -1, -1):
        prev, used = parents[e][cur]
        assign[e] = list(used)
        cur = prev
    return list(sizes), assign


def build_nc(sizes, reps=1):
    """Per-core program: yT[d, t] = wgt[t] * (silu(x @ w1) @ w2)[t, d]
    over len(sizes) weight slots."""
    bf16 = mybir.dt.bfloat16
    f32 = mybir.dt.float32
    ACT = mybir.ActivationFunctionType

    K = len(sizes)
    S = sum(sizes)
    nc = bacc.Bacc()
    xgt = nc.dram_tensor("xgt", [D, S], bf16, kind="ExternalInput")
    w1s = [
        nc.dram_tensor(f"w1_{k}", [D, DFF], bf16, kind="ExternalInput")
        for k in range(K)
    ]
    w2s = [
        nc.dram_tensor(f"w2_{k}", [DFF, D], bf16, kind="ExternalInput")
        for k in range(K)
    ]
    wgtb = nc.dram_tensor("wgtb", [P, S], f32, kind="ExternalInput")
    y = nc.dram_tensor("y", [D, S], f32, kind="ExternalOutput")

    xgt_r = xgt.rearrange("(k p) s -> p k s", p=P)  # [128, 8, S]
    w1_rs = [w.rearrange("(k p) f -> p k f", p=P) for w in w1s]
    w2_rs = [w.rearrange("(kf p) d -> p kf d", p=P) for w in w2s]
    y_r = y.rearrange("(m p) s -> m p s", p=P)  # [8, 128, S]

    # compile-time block schedule: (slot, tok0, B). Slot 0 starts with a
    # small block (its phase A is fused with block 1's, so the PE can start
    # after one small xg DMA + the first w1 chunk). The very last block is
    # small so the end-of-kernel drain waits on a short mult+DMA.
    sched = []
    off = 0
    for s, S_s in enumerate(sizes):
        t0 = off
        if s == 0 and S_s > 256:
            bs = [128] + _chunks_even(S_s - 128)
        else:
            bs = _chunks_even(S_s)
        if s == K - 1 and bs[-1] > 256:
            bs = bs[:-1] + [bs[-1] - 128, 128]
        for b in bs:
            sched.append((s, t0, b))
            t0 += b
        off += S_s

    with tile.TileContext(nc) as tc:
        with (
            tc.tile_pool(name="singles", bufs=1) as singles,
            tc.tile_pool(name="wres", bufs=1) as w_pool,
            tc.tile_pool(name="xg", bufs=3) as xg_pool,
            tc.tile_pool(name="ht", bufs=1) as h_pool,
            tc.tile_pool(name="ht0", bufs=1) as h0_pool,
            tc.tile_pool(name="yout", bufs=4) as y_pool,
            tc.tile_pool(name="hps", bufs=5, space="PSUM") as hpsum,
            tc.tile_pool(name="yps", bufs=3, space="PSUM") as ypsum,
        ):
            xg_tiles = {}  # i -> (tile, col0)

            def load_xg(i):
                _, tok0, B = sched[i]
                t = xg_pool.tile([P, KD, B], bf16, tag="xg", name="xg")
                nc.scalar.dma_start(out=t, in_=xgt_r[:, :, tok0 : tok0 + B])
                xg_tiles[i] = (t, 0)

            def load_weights(slot, xg_loader=None, wgt_late=False):
                """w1 chunks first (phase A streams them; graded sizes:
                small first so the PE starts early, large after for DMA
                efficiency), then w2 (needed in full only by the first
                phase B); wgtb early in the w2 stream. xg_loader: called
                right after w1's first chunk to slot in the first xg DMA
                (it gates the first matmuls; w1's chunk 0 gates only the
                Ldweights before them)."""
                w1_t = w_pool.tile([P, KD, DFF], bf16, tag="w1", name="w1")
                grades = [1, 1, 2, 4, 4, 4, 4, 4, 4, 4]
                mf = 0
                for gi, g in enumerate(grades):
                    nc.sync.dma_start(
                        out=w1_t[:, :, mf * P : (mf + g) * P],
                        in_=w1_rs[slot][:, :, mf * P : (mf + g) * P],
                    )
                    mf += g
                    if gi == 0 and xg_loader is not None:
                        xg_loader()
                assert mf == KF
                w2_t = w_pool.tile([P, KF, D], bf16, tag="w2", name="w2")
                for c4 in range(4):
                    nc.sync.dma_start(
                        out=w2_t[:, c4 * 8 : (c4 + 1) * 8, :],
                        in_=w2_rs[slot][:, c4 * 8 : (c4 + 1) * 8, :],
                    )
                    if c4 == 0 and wgt_late:
                        nc.sync.dma_start(out=wgt_t, in_=wgtb[:, :])
                return w1_t, w2_t

            def phase_a_multi(iis, w1_t, pools):
                """Fused phase A over several blocks: per-mf across all
                blocks, so each w1 chunk is consumed at the combined rate
                (lets the first blocks start before w1 fully lands)."""
                xs = [xg_tiles.pop(i) for i in iis]
                hts = []
                for i, pool in zip(iis, pools):
                    _, _, B = sched[i]
                    hts.append(
                        pool.tile(
                            [P, KF, B],
                            bf16,
                            tag="ht0" if pool is h0_pool else "ht",
                            name="ht",
                        )
                    )
                def emit(bi, mf):
                    i = iis[bi]
                    _, _, B = sched[i]
                    xt, c0 = xs[bi]
                    ph = hpsum.tile([P, B], f32, tag="hps", name="hps")
                    for kd in range(KD):
                        nc.tensor.matmul(
                            ph[:, :],
                            lhsT=w1_t[:, kd, mf * P : (mf + 1) * P],
                            rhs=xt[:, kd, c0 : c0 + B],
                            start=(kd == 0),
                            stop=(kd == KD - 1),
                        )
                    nc.scalar.activation(
                        hts[bi][:, mf, :], ph[:, :], ACT.Silu
                    )

                for mf in range(KF):
                    for bi in range(len(iis)):
                        emit(bi, mf)
                return hts

            def phase_a(i, w1_t, pool):
                return phase_a_multi([i], w1_t, [pool])[0]

            def phase_b(i, w2_t, hT):
                _, tok0, B = sched[i]
                for md in range(MD):
                    yp = ypsum.tile([P, B], f32, tag="yps", name="yps")
                    for kf in range(KF):
                        nc.tensor.matmul(
                            yp[:, :],
                            lhsT=w2_t[:, kf, md * P : (md + 1) * P],
                            rhs=hT[:, kf, :],
                            start=(kf == 0),
                            stop=(kf == KF - 1),
                        )
                    y_sb = y_pool.tile([P, B], f32, tag="yout", name="yout")
                    nc.vector.tensor_mul(
                        y_sb[:, :], yp[:, :], wgt_t[:, tok0 : tok0 + B]
                    )
                    nc.scalar.dma_start(
                        out=y_r[md, :, tok0 : tok0 + B], in_=y_sb[:, :]
                    )

            nblk = len(sched)
            wgt_t = singles.tile([P, S], f32)

            cur_slot = -1
            for rep in range(reps):
                i = 0
                while i < nblk:
                    slot, tok0, B = sched[i]
                    first = cur_slot == -1
                    if slot != cur_slot:
                        if first:
                            def ldr():
                                with tc.high_priority():
                                    load_xg(0)
                                    if nblk > 1:
                                        load_xg(1)
                        else:
                            ldr = None
                        w1_t, w2_t = load_weights(
                            slot, xg_loader=ldr, wgt_late=first
                        )
                        cur_slot = slot
                    fuse = (
                        rep == 0 and i == 0 and nblk > 1 and sched[1][0] == 0
                    )
                    if fuse:
                        if nblk > 2:
                            load_xg(2)
                        hT0, hT1 = phase_a_multi(
                            [0, 1], w1_t, [h0_pool, h_pool]
                        )
                        phase_b(0, w2_t, hT0)
                        phase_b(1, w2_t, hT1)
                        i = 2
                        continue
                    nxt = i + 1
                    if nxt < nblk:
                        if nxt not in xg_tiles:
                            load_xg(nxt)
                    elif rep + 1 < reps:
                        load_xg(0)
                    hT = phase_a(i, w1_t, h_pool)
                    phase_b(i, w2_t, hT)
                    i += 1

    if not nc.is_finalized():
        nc.finalize()
    return nc


def build_program(x, gate_w, w1, w2, top_k):
    import ml_dtypes

    x = np.asarray(x, dtype=np.float32)
    gate_w = np.asarray(gate_w, dtype=np.float32)
    w1 = np.asarray(w1, dtype=np.float32)
    w2 = np.asarray(w2, dtype=np.float32)
    assert int(top_k) == 2

    n = x.shape[0]
    ar = np.arange(n)

    # --- host routing (matches reference: softmax -> top2 -> renorm) ---
    logits = (x @ gate_w).astype(np.float64)
    i1 = np.argmax(logits, axis=1)
    lm = logits.copy()
    lm[ar, i1] = -np.inf
    i2 = np.argmax(lm, axis=1)
    m1 = logits[ar, i1]
    m2 = logits[ar, i2]
    g1 = 1.0 / (1.0 + np.exp(m2 - m1))  # = p1/(p1+p2)
    g2 = 1.0 - g1

    gw_full = np.zeros((n, E), dtype=np.float64)
    gw_full[ar, i1] = g1
    gw_full[ar, i2] = g2

    sel = np.zeros((n, E), dtype=bool)
    sel[ar, i1] = True
    sel[ar, i2] = True

    idxs = [np.nonzero(sel[:, e])[0] for e in range(E)]
    counts = np.array([len(ix) for ix in idxs])

    sizes, assign = _optimize_slots(counts)
    K = len(sizes)
    S = sum(sizes)
    bases = np.concatenate([[0], np.cumsum(sizes)]).astype(int)

    # --- bin placement: per slot class, 8 bins assigned to cores in order.
    class_bins = []  # class_bins[k][core] = expert or -1
    for k in range(K):
        lst = []
        for e in range(E):
            lst += [e] * assign[e][k]
        assert len(lst) <= E, (k, lst)
        lst += [-1] * (E - len(lst))
        class_bins.append(lst)

    # expert -> ordered list of (core, slot_base, capacity)
    exp_bins = {e: [] for e in range(E)}
    for k in range(K):
        for c, e in enumerate(class_bins[k]):
            if e >= 0:
                exp_bins[e].append((c, bases[k], sizes[k]))

    # token placement per expert: core_of[e][i], col_of[e][i]
    core_of = {}
    col_of = {}
    per_core_tokens = [[] for _ in range(E)]  # (col_base, tokens, wgts)
    for e in range(E):
        c_e = counts[e]
        core_arr = np.empty(c_e, dtype=np.int64)
        col_arr = np.empty(c_e, dtype=np.int64)
        pos = 0
        for core, base, cap in exp_bins[e]:
            take = min(c_e - pos, cap)
            if take <= 0:
                break
            core_arr[pos : pos + take] = core
            col_arr[pos : pos + take] = base + np.arange(take)
            toks = idxs[e][pos : pos + take]
            per_core_tokens[core].append(
                (base, toks, gw_full[toks, e].astype(np.float32))
            )
            pos += take
        assert pos == c_e, f"expert {e} not fully packed ({pos}/{c_e})"
        core_of[e] = core_arr
        col_of[e] = col_arr

    nc = build_nc(sizes, reps=REPS)

    bf = ml_dtypes.bfloat16
    in_maps = []
    for c in range(E):
        xg = np.zeros((S, D), dtype=np.float32)
        wg = np.zeros((S,), dtype=np.float32)
        for base, toks, wvals in per_core_tokens[c]:
            xg[base : base + len(toks)] = x[toks]
            wg[base : base + len(toks)] = wvals
        xgt = np.ascontiguousarray(xg.T).astype(bf)
        wgtb = np.broadcast_to(wg[None, :], (P, S)).copy()
        m = {"xgt": xgt, "wgtb": wgtb}
        for k in range(K):
            e_k = class_bins[k][c]
            e_k = e_k if e_k >= 0 else 0
            m[f"w1_{k}"] = w1[e_k].astype(bf)
            m[f"w2_{k}"] = w2[e_k].astype(bf)
        in_maps.append(m)

    meta = (i1, i2, core_of, col_of)
    return nc, in_maps, meta


def unshard(results, meta):
    i1, i2, core_of, col_of = meta
    n = len(i1)
    ysT = np.stack([results[c]["y"] for c in range(E)])  # [8, D, S]
    c1 = np.empty(n, dtype=np.int64)
    l1 = np.empty(n, dtype=np.int64)
    c2 = np.empty(n, dtype=np.int64)
    l2 = np.empty(n, dtype=np.int64)
    # core_of[e]/col_of[e] are aligned with expert e's ascending token list;
    # recover each token's position in that list via searchsorted.
    ar = np.arange(n)
    selm = np.zeros((n, E), dtype=bool)
    selm[ar, i1] = True
    selm[ar, i2] = True
    for e in range(E):
        toks = np.nonzero(selm[:, e])[0]
        p1 = np.searchsorted(toks, ar[i1 == e])
        c1[i1 == e] = core_of[e][p1]
        l1[i1 == e] = col_of[e][p1]
        p2 = np.searchsorted(toks, ar[i2 == e])
        c2[i2 == e] = core_of[e][p2]
        l2[i2 == e] = col_of[e][p2]
    y = ysT[c1, :, l1] + ysT[c2, :, l2]
    return y.astype(np.float32)


def kernel(x, gate_w, w1, w2, top_k):
    global LAST_RESULT
    nc, in_maps, meta = build_program(x, gate_w, w1, w2, top_k)
    try:
        res = run_bass_kernel_spmd(nc, in_maps, list(range(E)), trace=TRACE)
    except Exception:
        if not TRACE:
            raise
        # tracing unavailable in this environment; rerun untraced
        res = run_bass_kernel_spmd(nc, in_maps, list(range(E)), trace=False)
    global LAST_NC
    LAST_RESULT = res
    LAST_NC = nc
    return unshard(res.results, meta)
